# revision 22
# baseline (speedup 1.0000x reference)
"""GNN message-passing + pooling kernel for 8 Trainium2 NeuronCores.

Strategy (per the sharding hint):
  - Host: sort edges by dst, partition the 50k nodes into 8 contiguous
    ranges of 6250; each core gets the edges targeting its node range
    (disjoint scatter -> no cross-core reduction needed).
  - Host pads each node's edge list to even length and lays edges out as
    PAIRS sharing a dst: pair p of a supertile occupies columns p and
    p+256.  Host gathers x[dst], x[src], edge_attr into a transposed
    bf16 [320, E_pad] tensor per core (pairs grouped into 481-node
    scatter windows, padded to a uniform supertile count CP so the
    device program is identical across cores).
  - Device (per core): 4-layer message MLP in transposed-activation
    layout, weight-stationary over blocks of 6 supertiles.  The
    remainder K-chunks (44/64 rows) of each layer are row-packed: two
    adjacent supertiles' matmuls run concurrently in disjoint PE row
    groups (tile_position 0 / 64), using duplicated weight/activation
    rows.  After layer 3, h3 is summed over edge pairs (L4 and the
    scatter are linear in h3), halving L4 + scatter work.  Scatter-add
    uses one-hot matmuls into a per-window PSUM accumulator; the
    window's rank-1 corrections (deg (x) mb4 for the L4 bias and
    -odd (x) msg0 to cancel the padding member of odd pairs) are a
    single K=2 matmul.  Node MLP + per-graph sum-pooling accumulate in
    PSUM.  Output: [32, 128] partial per-graph sums.
  - Host: sum the 8 partials, add counts*nb4, divide by per-graph node
    counts, apply the final [128, 16] linear.
"""

import sys

if "/opt/trn_rl_repo" not in sys.path:
    sys.path.insert(0, "/opt/trn_rl_repo")

import numpy as np
import ml_dtypes

BF16 = ml_dtypes.bfloat16

# Problem dims
N_NODES = 50000
N_EDGES = 800000
NF = 128          # node feature dim
EF = 64           # edge feature dim
MSGD = 128        # message dim
HID = 300         # MLP hidden
G = 32            # graphs
NCORES = 8

# Tiling config
NPC = N_NODES // NCORES   # 6250 nodes per core
NW = 481                  # nodes per scatter window
W = 13                    # windows per core (13*481 = 6253 >= 6250)
ST = 512                  # edge supertile (free dim per matmul)
PPS = ST // 2             # pairs per supertile (256)
NP2 = 6656                # padded nodes per core for node MLP (13*512)
NT = NP2 // ST            # node supertiles
NCHK = NP2 // 128         # node chunks for pooling
BMAX = 6                  # supertiles per weight-stationary block

TRACE = False             # set True from test harness to profile core 0
LAST_EXEC_NS = None
LAST_RES = None

_BUILD_CACHE = {}


def _chunks(total, step=128):
    return [(o, min(step, total - o)) for o in range(0, total, step)]


def _blocks(n, bmax=BMAX):
    out = []
    o = 0
    while o < n:
        b = min(bmax, n - o)
        out.append((o, b))
        o += b
    return out


def _build_nc(CP):
    """Build the (single) SPMD Bass program. CP = supertiles per window."""
    import concourse.bacc as bacc
    import concourse.tile as tile
    from concourse import mybir
    from contextlib import ExitStack

    f32 = mybir.dt.float32
    bf16 = mybir.dt.bfloat16
    AF = mybir.ActivationFunctionType
    OP = mybir.AluOpType

    E_pad = W * CP * ST
    NCH = W * CP * 2          # pair chunks (2 per supertile)

    nc = bacc.Bacc("TRN2", target_bir_lowering=False, debug=False,
                   num_devices=NCORES)

    # --- DRAM I/O ---
    d_msg_inT = nc.dram_tensor("msg_inT", [2 * NF + EF, E_pad], bf16,
                               kind="ExternalInput")
    d_dstloc = nc.dram_tensor("dstloc", [128, NCH], f32,
                              kind="ExternalInput")
    d_xT = nc.dram_tensor("xT", [NF, NP2], bf16, kind="ExternalInput")
    d_pmat = nc.dram_tensor("pmat", [128, NCHK * G], bf16,
                            kind="ExternalInput")
    d_dego = nc.dram_tensor("dego", [2, W * NW], bf16, kind="ExternalInput")
    d_bias2 = nc.dram_tensor("bias2", [2, MSGD], bf16, kind="ExternalInput")
    d_mW = [nc.dram_tensor(f"mW{i}", s, bf16, kind="ExternalInput")
            for i, s in enumerate([[2 * NF + EF, HID], [HID, HID], [HID, HID],
                                   [HID, MSGD]], start=1)]
    d_mb = [nc.dram_tensor(f"mb{i}", [HID, 1], f32, kind="ExternalInput")
            for i in range(1, 4)]
    d_nW = [nc.dram_tensor(f"nW{i}", s, bf16, kind="ExternalInput")
            for i, s in enumerate([[NF + MSGD, HID], [HID, HID], [HID, HID],
                                   [HID, NF]], start=1)]
    d_nb = [nc.dram_tensor(f"nb{i}", [HID, 1], f32, kind="ExternalInput")
            for i in range(1, 4)]
    d_out = nc.dram_tensor("partial", [G, NF], f32, kind="ExternalOutput")

    HCH = _chunks(HID)          # [(0,128),(128,128),(256,44)]
    KIN = _chunks(2 * NF + EF)  # [(0,128),(128,128),(256,64)]

    with tile.TileContext(nc) as tc, ExitStack() as ctx:
        wpool = ctx.enter_context(tc.tile_pool(name="w", bufs=1))
        apool = ctx.enter_context(tc.tile_pool(name="agg", bufs=1))
        inpool = ctx.enter_context(tc.tile_pool(name="in", bufs=2))
        hpool = ctx.enter_context(tc.tile_pool(name="h", bufs=2))
        ppool = ctx.enter_context(tc.tile_pool(name="p3", bufs=3))
        mpool = ctx.enter_context(tc.tile_pool(name="m", bufs=3))
        spool = ctx.enter_context(tc.tile_pool(name="s", bufs=8))
        mm_psum = ctx.enter_context(
            tc.tile_pool(name="mmp", bufs=7, space="PSUM"))
        acc_psum = ctx.enter_context(
            tc.tile_pool(name="accp", bufs=1, space="PSUM"))

        def load_w(dram, K, N, dt, name, dup_last=False):
            """Load K-chunked weight tiles; if dup_last, the last (<=64 row)
            chunk is stored twice: at partitions 0.. and 64.."""
            tiles = []
            ch = _chunks(K)
            for i, (k0, kk) in enumerate(ch):
                if dup_last and i == len(ch) - 1 and kk <= 64:
                    t = wpool.tile([64 + kk, N], dt, tag=f"{name}{i}")
                    nc.sync.dma_start(t[:kk, :], dram[k0:k0 + kk, :])
                    nc.sync.dma_start(t[64:64 + kk, :], dram[k0:k0 + kk, :])
                else:
                    t = wpool.tile([kk, N], dt, tag=f"{name}{i}")
                    nc.sync.dma_start(t[:, :], dram[k0:k0 + kk, :])
                tiles.append(t)
            return tiles

        mW = [load_w(d_mW[0], 2 * NF + EF, HID, bf16, "mW1", dup_last=True),
              load_w(d_mW[1], HID, HID, bf16, "mW2", dup_last=True),
              load_w(d_mW[2], HID, HID, bf16, "mW3", dup_last=True),
              load_w(d_mW[3], HID, MSGD, bf16, "mW4")]
        mb = [load_w(d_mb[i], HID, 1, f32, f"mb{i + 1}") for i in range(3)]
        nW = [load_w(d_nW[0], NF + MSGD, HID, bf16, "nW1"),
              load_w(d_nW[1], HID, HID, bf16, "nW2", dup_last=True),
              load_w(d_nW[2], HID, HID, bf16, "nW3", dup_last=True),
              load_w(d_nW[3], HID, NF, bf16, "nW4")]
        nb = [load_w(d_nb[i], HID, 1, f32, f"nb{i + 1}") for i in range(3)]
        bias2 = wpool.tile([2, MSGD], bf16, tag="bias2")
        nc.sync.dma_start(bias2[:, :], d_bias2[:, :])
        dego = wpool.tile([2, W * NW], bf16, tag="dego")
        nc.sync.dma_start(dego[:, :], d_dego[:, :])

        dstloc = wpool.tile([128, NCH], f32, tag="dstloc")
        nc.sync.dma_start(dstloc[:, :], d_dstloc[:, :])
        xT = wpool.tile([NF, NP2], bf16, tag="xT")
        nc.gpsimd.dma_start(xT[:, :], d_xT[:, :])
        pmat = wpool.tile([128, NCHK * G], bf16, tag="pmat")
        nc.gpsimd.dma_start(pmat[:, :], d_pmat[:, :])

        iota = wpool.tile([128, NW], mybir.dt.float16, tag="iota")
        nc.gpsimd.iota(iota[:, :], pattern=[[1, NW]], base=0,
                       channel_multiplier=0,
                       allow_small_or_imprecise_dtypes=True)

        aggrT = apool.tile([NF, NP2], bf16, tag="aggrT")
        # scatter windows cover cols [0, W*NW); zero the tail
        nc.gpsimd.memset(aggrT[:, W * NW:NP2], 0.0)

        # counter for alternating act-engine assignment
        eng_flip = [0]

        def act_relu(dst, src, bias):
            eng_flip[0] = (eng_flip[0] + 1) % 5
            if eng_flip[0] < 3:
                nc.scalar.activation(dst, src, AF.Relu, bias=bias)
            else:
                nc.vector.tensor_scalar(dst, src, bias, 0.0,
                                        op0=OP.add, op1=OP.max)

        def front(ksrc0, Bb, Wt, bt, dup_in):
            """3 weight-stationary MLP layers over Bb supertiles.

            ksrc0: list of (tile, colbase, kk, hi_base) layer-1 K-chunks;
            hi_base is the partition base of the duplicated rows (or None).
            Returns h3 tiles (list of (tile, kk)), block-local cols.
            """
            cur = ksrc0
            h_out = None
            ntri = (Bb + 2) // 3
            for layer in range(3):
                tag = ("A", "B", "A")[layer]
                h_out = []
                for m, (m0, mm_) in enumerate(HCH):
                    pst = [mm_psum.tile([128, ST], mybir.dt.float32,
                                        tag="mm", name=f"ps{g}")
                           for g in range(Bb)]

                    def ps(g, mm=None):
                        return pst[g][:mm, :]

                    nk = len(cur)
                    has_rem = cur[-1][3] is not None
                    nfull = nk - 1 if has_rem else nk
                    ht = hpool.tile([128, BMAX * ST], bf16,
                                    tag=f"h{tag}{m}")

                    def emit_conv(g):
                        act_relu(ht[:mm_, g * ST:(g + 1) * ST],
                                 ps(g, mm_), bt[layer][m][:mm_, :])
                        # layers 1-2 feed the next layer's row-packed
                        # remainder chunk: odd supertiles need the rows
                        # duplicated at partition 64 (DVE cross-quadrant
                        # write: 44-part op, src Q0/Q1 -> dst Q2/Q3)
                        if layer < 2 and m == 2 and g % 2 == 1:
                            nc.vector.tensor_copy(
                                ht[64:64 + mm_, g * ST:(g + 1) * ST],
                                ht[:mm_, g * ST:(g + 1) * ST])

                    # k-outer keeps each weight tile resident for Bb
                    # consecutive matmuls; the remainder round then closes
                    # each bank pair and issues its conversion immediately
                    for k in range(nfull):
                        kt, kb, kk, _ = cur[k]
                        wt = Wt[layer][k]
                        for g in range(Bb):
                            nc.tensor.matmul(
                                ps(g, mm_), wt[:kk, m0:m0 + mm_],
                                kt[:kk, kb + g * ST:kb + (g + 1) * ST],
                                start=(k == 0), stop=(k == nk - 1))
                    for gp in range(0, Bb, 2):
                        if has_rem:
                            kt, kb, kk, hib = cur[-1]
                            wt = Wt[layer][nk - 1]
                            # row-packed remainder: adjacent supertiles in
                            # disjoint PE row groups (0 and 64)
                            nc.tensor.matmul(
                                ps(gp, mm_), wt[:kk, m0:m0 + mm_],
                                kt[:kk, kb + gp * ST:kb + (gp + 1) * ST],
                                start=False, stop=True)
                            if gp + 1 < Bb:
                                nc.tensor.matmul(
                                    ps(gp + 1, mm_),
                                    wt[hib:hib + kk, m0:m0 + mm_],
                                    kt[hib:hib + kk,
                                       kb + (gp + 1) * ST:kb + (gp + 2) * ST],
                                    start=False, stop=True)
                        emit_conv(gp)
                        if gp + 1 < Bb:
                            emit_conv(gp + 1)
                    h_out.append(ht)
                cur = [(h_out[i], 0, kk, 64 if i == len(HCH) - 1 else None)
                       for i, (_, kk) in enumerate(HCH)]
            return [(h_out[i], kk) for i, (_, kk) in enumerate(HCH)]

        # ================= edge phase =================
        for w in range(W):
            accp = acc_psum.tile([128, ST], mybir.dt.float32, tag="acc")
            # rank-1 corrections: aggr += mb4 (x) deg - msg0 (x) odd
            nc.tensor.matmul(accp[:MSGD, :NW], bias2[:, :],
                             dego[:, w * NW:(w + 1) * NW],
                             start=True, stop=False, skip_group_check=True)
            pending = None

            def do_scatter(p):
                msgt, cidx0, last = p
                for e in range(2):
                    cidx = cidx0 + e
                    st = spool.tile([128, NW], bf16, tag="S")
                    nc.vector.tensor_scalar(
                        st[:, :], iota[:, :], dstloc[:, cidx:cidx + 1], None,
                        op0=OP.is_equal)
                    nc.tensor.matmul(accp[:MSGD, :NW],
                                     msgt[:, e * 128:(e + 1) * 128], st[:, :],
                                     start=False, stop=(last and e == 1),
                                     skip_group_check=True)

            for s0, Bb in _blocks(CP):
                base = (w * CP + s0) * ST
                in_t = []
                for i, (k0, kk) in enumerate(KIN):
                    dup = (i == len(KIN) - 1)
                    t = inpool.tile([128 if dup else kk, BMAX * ST], bf16,
                                    tag=f"in{i}")
                    nc.sync.dma_start(
                        t[:kk, :Bb * ST],
                        d_msg_inT[k0:k0 + kk, base:base + Bb * ST])
                    if dup:
                        nc.sync.dma_start(
                            t[64:64 + kk, :Bb * ST],
                            d_msg_inT[k0:k0 + kk, base:base + Bb * ST])
                    in_t.append((t, 0, kk, 64 if dup else None))
                h3 = front(in_t, Bb, mW, mb, True)
                for g in range(Bb):
                    s = s0 + g
                    # sum h3 over edge pairs (cols j and j+256 share a dst)
                    h3p = []
                    for k, (k0, kk) in enumerate(HCH):
                        hp = ppool.tile([kk, PPS], bf16, tag=f"h3p{k}",
                                        name=f"h3p{k}")
                        nc.vector.tensor_add(
                            hp[:, :], h3[k][0][:kk, g * ST:g * ST + PPS],
                            h3[k][0][:kk, g * ST + PPS:(g + 1) * ST])
                        h3p.append(hp)
                    mp = mm_psum.tile([128, ST], mybir.dt.float32,
                                      tag="mm", name="mp")
                    for e in range(2):
                        for k, (k0, kk) in enumerate(HCH):
                            nc.tensor.matmul(
                                mp[:, e * 128:(e + 1) * 128],
                                h3p[k][:kk, e * 128:(e + 1) * 128],
                                mW[3][k][:, :], start=(k == 0),
                                stop=(k == len(HCH) - 1))
                    msgt = mpool.tile([128, PPS], bf16, tag="msg")
                    nc.scalar.activation(msgt[:, :], mp[:, :PPS], AF.Copy)
                    if pending is not None:
                        do_scatter(pending)
                    pending = (msgt, (w * CP + s) * 2, s == CP - 1)
            do_scatter(pending)
            nc.vector.tensor_copy(aggrT[:, w * NW:(w + 1) * NW],
                                  accp[:MSGD, :NW])

        # ================= node phase =================
        accn = acc_psum.tile([128, ST], mybir.dt.float32, tag="acc")
        pendn = None

        def do_pool(p):
            no, t = p
            for e in range(4):
                tch = t * 4 + e
                nc.tensor.matmul(accn[:G, :NF],
                                 pmat[:, tch * G:(tch + 1) * G],
                                 no[:, e * 128:(e + 1) * 128],
                                 start=(tch == 0), stop=(tch == NCHK - 1),
                                 skip_group_check=True)

        for s0, Bb in _blocks(NT, 5):
            ksrc0 = [(xT, s0 * ST, NF, None), (aggrT, s0 * ST, MSGD, None)]
            h3 = front(ksrc0, Bb, nW, nb, False)
            for g in range(Bb):
                t = s0 + g
                np_ = mm_psum.tile([128, ST], mybir.dt.float32,
                                   tag="mm", name="np_")
                for e in range(4):
                    for k, (k0, kk) in enumerate(HCH):
                        nc.tensor.matmul(
                            np_[:, e * 128:(e + 1) * 128],
                            h3[k][0][:kk, (g * 4 + e) * 128:
                                     (g * 4 + e + 1) * 128],
                            nW[3][k][:, :], start=(k == 0),
                            stop=(k == len(HCH) - 1))
                no = mpool.tile([128, ST], bf16, tag="msg")
                nc.scalar.activation(no[:, :], np_[:, :], AF.Copy)
                if pendn is not None:
                    do_pool(pendn)
                pendn = (no, t)
        do_pool(pendn)

        pooled = apool.tile([G, NF], f32, tag="pooled")
        nc.vector.tensor_copy(pooled[:, :], accn[:G, :NF])
        nc.sync.dma_start(d_out[:, :], pooled[:, :])

    nc.compile()
    return nc


def _msg0_bf16(weights):
    """Device-faithful message-MLP output for an all-zero input column
    (no final bias): h=relu(b); 2x (relu(h@W+b)); h@W4 -- bf16 activations,
    f32 accumulation, matching the kernel's layer pipeline."""
    h = np.maximum(np.asarray(weights["mb1"], np.float32), 0).astype(BF16)
    for l in (2, 3):
        Wl = np.asarray(weights[f"mW{l}"], np.float32).astype(BF16)
        z = h.astype(np.float32) @ Wl.astype(np.float32) + np.asarray(
            weights[f"mb{l}"], np.float32)
        h = np.maximum(z, 0).astype(BF16)
    W4 = np.asarray(weights["mW4"], np.float32).astype(BF16)
    return h.astype(np.float32) @ W4.astype(np.float32)  # [MSGD] f32


def _prep_inputs(x, edge_index, edge_attr, batch, weights, CP):
    """Host-side shard/gather/pair/pad. Returns per-core in_maps."""
    E_pad = W * CP * ST
    src = np.asarray(edge_index[0], np.int64)
    dst = np.asarray(edge_index[1], np.int64)

    order = np.argsort(dst, kind="stable")
    dsts = dst[order]
    srcs = src[order]

    xT = np.ascontiguousarray(np.asarray(x, np.float32).astype(BF16).T)
    eaT = np.ascontiguousarray(np.asarray(edge_attr, np.float32).astype(BF16).T)
    batch = np.asarray(batch, np.int64)

    bounds = np.searchsorted(dsts, np.arange(0, N_NODES + 1, NPC))

    wcommon = {}
    for i in range(1, 5):
        wcommon[f"mW{i}"] = np.ascontiguousarray(
            weights[f"mW{i}"].astype(BF16))
        wcommon[f"nW{i}"] = np.ascontiguousarray(
            weights[f"nW{i}"].astype(BF16))
    for i in range(1, 4):
        wcommon[f"mb{i}"] = np.ascontiguousarray(
            weights[f"mb{i}"].reshape(HID, 1).astype(np.float32))
        wcommon[f"nb{i}"] = np.ascontiguousarray(
            weights[f"nb{i}"].reshape(HID, 1).astype(np.float32))
    msg0 = _msg0_bf16(weights)
    wcommon["bias2"] = np.ascontiguousarray(np.stack(
        [weights["mb4"].astype(np.float32), -msg0]).astype(BF16))

    garange = np.arange(G)
    in_maps = []
    for k in range(NCORES):
        sl = slice(int(bounds[k]), int(bounds[k + 1]))
        eidx = order[sl]
        dloc = dsts[sl] - k * NPC
        srck = srcs[sl]

        deg = np.bincount(dloc, minlength=NPC)          # per local node
        cumd = np.concatenate([[0], np.cumsum(deg)])
        P_n = (deg + 1) // 2                            # pairs per node
        cumP = np.concatenate([[0], np.cumsum(P_n)])
        winstart = cumP[np.arange(W) * NW]              # pair base per window
        win_n = np.arange(NPC) // NW
        pairbase = cumP[:-1] - winstart[win_n]          # window-local

        r = np.arange(len(dloc)) - cumd[dloc]           # rank within node
        q = pairbase[dloc] + r // 2                     # window-local pair
        wj = dloc // NW
        s_j = q // PPS
        col = q % PPS
        pos = (wj * CP + s_j) * ST + col + PPS * (r % 2)

        msg_inT = np.zeros((2 * NF + EF, E_pad), BF16)
        msg_inT[0:NF, pos] = xT[:, k * NPC + dloc]
        msg_inT[NF:2 * NF, pos] = xT[:, srck]
        msg_inT[2 * NF:, pos] = eaT[:, eidx]

        # dst of each pair slot (-1 = empty)
        dl = np.full(W * CP * PPS, -1.0, np.float32)
        rep = r % 2 == 0
        dl[wj[rep] * CP * PPS + q[rep]] = (dloc - wj * NW)[rep]
        dstloc = np.ascontiguousarray(dl.reshape(W * CP * 2, 128).T)

        xTn = np.zeros((NF, NP2), BF16)
        xTn[:, :NPC] = xT[:, k * NPC:(k + 1) * NPC]

        dego = np.zeros((2, W * NW), BF16)
        dego[0, :] = np.bincount(dloc, minlength=W * NW).astype(BF16)
        odd = np.zeros(W * NW, np.float32)
        odd[:NPC] = deg % 2
        dego[1, :] = odd.astype(BF16)

        bl = np.full(NP2, -1, np.int64)
        bl[:NPC] = batch[k * NPC:(k + 1) * NPC]
        P = (bl[:, None] == garange[None, :]).astype(BF16)
        pmat = np.ascontiguousarray(
            P.reshape(NCHK, 128, G).transpose(1, 0, 2).reshape(128, NCHK * G))

        in_map = dict(wcommon)
        in_map.update(msg_inT=msg_inT, dstloc=dstloc, xT=xTn, pmat=pmat,
                      dego=dego)
        in_maps.append(in_map)
    return in_maps


def kernel(**inputs):
    global LAST_EXEC_NS, LAST_RES
    from concourse.bass_utils import run_bass_kernel_spmd

    x = np.asarray(inputs["x"], np.float32)
    edge_index = np.asarray(inputs["edge_index"])
    edge_attr = np.asarray(inputs["edge_attr"], np.float32)
    batch = np.asarray(inputs["batch"])

    # supertiles per window from the actual data (uniform across cores):
    # pairs per (core, window) after padding each node's edges to even
    dst = np.asarray(edge_index[1], np.int64)
    deg = np.bincount(dst, minlength=N_NODES)
    pairs = (deg + 1) // 2
    wid = (np.arange(N_NODES) // NPC) * W + (np.arange(N_NODES) % NPC) // NW
    pw = np.bincount(wid, weights=pairs, minlength=NCORES * W)
    CP = max(2, int(np.ceil(pw.max() / PPS)))

    key = CP
    if key not in _BUILD_CACHE:
        _BUILD_CACHE[key] = _build_nc(CP)
    nc = _BUILD_CACHE[key]

    in_maps = _prep_inputs(x, edge_index, edge_attr, batch, inputs, CP)

    res = run_bass_kernel_spmd(nc, in_maps, list(range(NCORES)), trace=TRACE)
    LAST_EXEC_NS = res.exec_time_ns
    LAST_RES = res

    total = np.zeros((G, NF), np.float64)
    for r in res.results:
        total += np.asarray(r["partial"], np.float64)

    counts = np.bincount(np.asarray(batch, np.int64), minlength=G)
    # node-MLP L4 bias was dropped on device; fold it in here
    total += counts[:, None].astype(np.float64) * np.asarray(
        inputs["nb4"], np.float64)[None, :]
    pooled = (total / np.maximum(counts, 1)[:, None]).astype(np.float32)
    out = pooled @ np.asarray(inputs["linW"], np.float32) + np.asarray(
        inputs["linb"], np.float32)
    return out.astype(np.float32)
